# revision 5
# baseline (speedup 1.0000x reference)
"""Multi-head attention kernel for Trainium2 (8 NeuronCores).

Problem: B=4, T=2048, U=1024, H=16 heads, D=64. Full (non-causal) softmax
attention per head. 64 independent (head, batch) problems.

Sharding: core c owns batch b = c//2 and head block hb = c%2 (8 contiguous
heads = 512 contiguous channels). No cross-core communication.

Per-core algorithm (T=2048, DD=512 channels, 8 local heads of D=64):
  - Q, K are loaded, converted to bf16 on GpSimd, bounced through a DRAM
    scratch, and transpose-loaded (DMA xbar) into QT/KT [d, t] layout,
    two heads per 128-partition tile.
  - V is converted to bf16 into a per-t-chunk [128, 8*65] layout where each
    head's 64 columns are augmented with a ones column (computes the
    softmax denominator for free in the second matmul).
  - For each head h, q-half qh (1024 q), k-chunk kc (128 k):
      mm1: scoresT[k, q] = KT_chunk.T @ QT  (PSUM, fp32)
      exp: probsT = exp(scoresT / 8)        (ACT, bf16 -> SBUF)
      mm2: outT[65, q] += V_aug[kc].T @ probsT  (PSUM accumulate, fp32)
    Then normalize: r = 1/outT[64]; broadcast across partitions (GpSimd);
    out = outT[0:64] * r; split fp32 into bf16 hi+lo; DMA-transpose both to
    [q, d] orientation; DVE re-add to fp32; store.
"""

import os
import sys

sys.path.insert(0, "/opt/trn_rl_repo")

import numpy as np

import concourse.bass as bass
import concourse.bacc as bacc
import concourse.mybir as mybir
import concourse.tile as tile
from concourse import library_config
from concourse.bass_utils import run_bass_kernel_spmd

F32 = mybir.dt.float32
BF16 = mybir.dt.bfloat16
EXP = mybir.ActivationFunctionType.Exp

B, T, U = 4, 2048, 1024
H_TOTAL, D = 16, 64
DD = 512          # channels per core (8 heads)
H = 8             # heads per core
NQ = 1024         # q-half size
KC = 16           # k chunks of 128
TC = 16           # t chunks of 128
HP = 4            # head pairs
N_CORES = 8
SCALE = 1.0 / 8.0  # 1/sqrt(D)


def build_program(nc):
    q_d = nc.dram_tensor("querys", [T, DD], F32, kind="ExternalInput").ap()
    k_d = nc.dram_tensor("keys", [T, DD], F32, kind="ExternalInput").ap()
    v_d = nc.dram_tensor("values", [T, DD], F32, kind="ExternalInput").ap()
    o_d = nc.dram_tensor("out", [T, DD], F32, kind="ExternalOutput").ap()
    qbf_d = nc.dram_tensor("qbf_scratch", [T, DD], BF16).ap()
    kbf_d = nc.dram_tensor("kbf_scratch", [T, DD], BF16).ap()

    with tile.TileContext(nc) as tc:
        with (
            tc.tile_pool(name="persist", bufs=1) as persist,
            tc.tile_pool(name="stage", bufs=4) as stage,
            tc.tile_pool(name="probs", bufs=3) as probs_pool,
            tc.tile_pool(name="norm", bufs=2) as norm_pool,
            tc.tile_pool(name="ps_sc", bufs=2, space=bass.MemorySpace.PSUM) as ps_sc,
            tc.tile_pool(name="ps_o", bufs=2, space=bass.MemorySpace.PSUM) as ps_o,
        ):
            nc.gpsimd.load_library(library_config.attn)

            # Dummy exp to hoist the ACT table load to t=0.
            warm = persist.tile([1, 1], F32, tag="warm")
            nc.gpsimd.memset(warm[:], 0.0)
            warm_o = persist.tile([1, 1], F32, tag="warm_o")
            nc.scalar.activation(warm_o[:], warm[:], EXP)

            # ---- V preprocessing: fp32 -> bf16 with ones-augmented layout ----
            vc = []
            for c in range(TC):
                vt = persist.tile([128, H * 65], BF16, tag=f"vc{c}", name=f"vc{c}")
                vc.append(vt)
                nc.gpsimd.memset(
                    vt[:].rearrange("p (h e) -> p h e", e=65)[:, :, 64:65], 1.0
                )
            v_3d = v_d.rearrange("(c p) d -> c p d", p=128)
            for c in range(TC):
                vs = stage.tile([128, DD], F32, tag="vstage")
                nc.sync.dma_start(vs[:], v_3d[c])
                nc.gpsimd.tensor_copy(
                    vc[c][:].rearrange("p (h e) -> p h e", e=65)[:, :, 0:64],
                    vs[:].rearrange("p (h e) -> p h e", e=64),
                )

            # ---- Q/K preprocessing: convert to bf16 scratch, transpose-load ----
            qt = [persist.tile([128, T], BF16, tag=f"qt{hp}", name=f"qt{hp}") for hp in range(HP)]
            kt = [persist.tile([128, T], BF16, tag=f"kt{hp}", name=f"kt{hp}") for hp in range(HP)]
            for src_d, dst_d in ((q_d, qbf_d), (k_d, kbf_d)):
                src3 = src_d.rearrange("(c p) d -> c p d", p=128)
                dst3 = dst_d.rearrange("(c p) d -> c p d", p=128)
                for c in range(TC):
                    s = stage.tile([128, DD], F32, tag="qkstage")
                    nc.sync.dma_start(s[:], src3[c])
                    sb = stage.tile([128, DD], BF16, tag="qkbf")
                    nc.gpsimd.tensor_copy(sb[:], s[:])
                    nc.sync.dma_start(dst3[c], sb[:])
            for hp in range(HP):
                for th in range(2):  # t halves so compute can start earlier
                    tsl = slice(th * 1024, (th + 1) * 1024)
                    csl = slice(hp * 128, (hp + 1) * 128)
                    nc.sync.dma_start(
                        qt[hp][:, tsl], qbf_d[tsl, csl], transpose=True
                    )
                    nc.sync.dma_start(
                        kt[hp][:, tsl], kbf_d[tsl, csl], transpose=True
                    )

            # ---- main loop ----
            for h in range(H):
                hp, base = h // 2, (h % 2) * 64
                for qh in range(2):
                    outp = ps_o.tile([65, NQ], F32, tag="outp")
                    for kc in range(KC):
                        sc = ps_sc.tile([128, NQ], F32, tag="sc")
                        lhsT = kt[hp][base : base + 64, kc * 128 : (kc + 1) * 128]
                        for j in range(2):
                            nc.tensor.matmul(
                                sc[:, j * 512 : (j + 1) * 512],
                                lhsT,
                                qt[hp][
                                    base : base + 64,
                                    qh * NQ + j * 512 : qh * NQ + (j + 1) * 512,
                                ],
                                start=True,
                                stop=True,
                            )
                        pb = probs_pool.tile([128, NQ], BF16, tag="pb")
                        nc.scalar.activation(pb[:], sc[:], EXP, scale=SCALE)
                        vsl = vc[kc][:, h * 65 : (h + 1) * 65]
                        for j in range(2):
                            nc.tensor.matmul(
                                outp[:, j * 512 : (j + 1) * 512],
                                vsl,
                                pb[:, j * 512 : (j + 1) * 512],
                                start=(kc == 0),
                                stop=(kc == KC - 1),
                            )
                    # normalize + transpose + store
                    r = norm_pool.tile([1, NQ], F32, tag="r")
                    nc.vector.reciprocal(r[:], outp[64:65, :])
                    bc = norm_pool.tile([64, NQ], F32, tag="bc")
                    nc.gpsimd.partition_broadcast(bc[:], r[:])
                    ob = norm_pool.tile([64, NQ], F32, tag="ob")
                    nc.vector.tensor_mul(ob[:], outp[0:64, :], bc[:])
                    hi = norm_pool.tile([64, NQ], BF16, tag="hi")
                    nc.vector.tensor_copy(hi[:], ob[:])
                    lo = norm_pool.tile([64, NQ], BF16, tag="lo")
                    nc.vector.tensor_sub(lo[:], ob[:], hi[:])
                    hi_t = norm_pool.tile([128, 512], BF16, tag="hi_t")
                    lo_t = norm_pool.tile([128, 512], BF16, tag="lo_t")
                    nc.sync.dma_start(
                        hi_t[:].rearrange("p (m l) -> p m l", l=64),
                        hi[:],
                        transpose=True,
                    )
                    nc.sync.dma_start(
                        lo_t[:].rearrange("p (m l) -> p m l", l=64),
                        lo[:],
                        transpose=True,
                    )
                    ob2 = norm_pool.tile([128, 512], F32, tag="ob2")
                    nc.vector.tensor_add(ob2[:], hi_t[:], lo_t[:])
                    # dest: out[qh*1024 + m*128 + p, h*64 + d] <- ob2[p, m*64+d]
                    dest = o_d[
                        qh * NQ : (qh + 1) * NQ, h * 64 : (h + 1) * 64
                    ].rearrange("(m p) d -> p m d", p=128)
                    nc.gpsimd.dma_start(dest, ob2[:].rearrange("p (m l) -> p m l", l=64))
    return nc


_CACHED = None


def _get_program():
    global _CACHED
    if _CACHED is None:
        nc = bacc.Bacc("TRN2", target_bir_lowering=False, debug=False)
        _CACHED = build_program(nc)
        _CACHED.compile()
    return _CACHED


def _make_in_maps(querys, keys, values):
    querys = np.ascontiguousarray(np.asarray(querys, dtype=np.float32))
    keys = np.ascontiguousarray(np.asarray(keys, dtype=np.float32))
    values = np.ascontiguousarray(np.asarray(values, dtype=np.float32))
    in_maps = []
    for c in range(N_CORES):
        b, hb = c // 2, c % 2
        sl = slice(hb * DD, (hb + 1) * DD)
        in_maps.append(
            {
                "querys": querys[b, :, sl],
                "keys": keys[b, :, sl],
                "values": values[b, :, sl],
            }
        )
    return in_maps


def kernel(querys, keys, values):
    nc = _get_program()
    in_maps = _make_in_maps(querys, keys, values)
    res = run_bass_kernel_spmd(nc, in_maps, list(range(N_CORES)))
    out = np.empty((B, T, U), dtype=np.float32)
    for c in range(N_CORES):
        b, hb = c // 2, c % 2
        out[b, :, hb * DD : (hb + 1) * DD] = res.results[c]["out"]
    return out
